# revision 15
# baseline (speedup 1.0000x reference)
"""v10: bf16 conv; jitter-robust head schedule.

Queue start order flips run to run (+-1us), so no tight cross-queue
deadlines: w loads as 3 k-group slices leading the sync queue (9 per-k
slices lost 5.6us to serialized ~620ns DMA triggers on the sync
engine); t=0 interleaves h0/h1 k-groups so w[3:6]/w[6:9] aren't needed
until ~1.1/2.2us after mm0. All x row-tiles + b lead the scalar queue
(xa00 first; only it gates mm0). Warmup 11 matmuls ~= the 3us p-state
ramp the PE needs after body entry anyway; any PE idle gap costs ~2x
(re-ramp), so warmup errs long.

v12: H-edge tap elision. Out row 55 only receives kh=0,1 (kh=2 reads
the zero pad row): t6 computes as a 7-row 9-tap piece + 1-row 6-tap
piece. v14: the two pieces drain as two bias-adds but ONE 448-col out
DMA (tiny trailing DMAs cost ~1.8us trigger-to-sem each); outputs are
bf16 (halved HBM write traffic/power), upcast to f32 on host.
"""

import sys

if "/opt/trn_rl_repo" not in sys.path:
    sys.path.insert(0, "/opt/trn_rl_repo")

import ml_dtypes
import numpy as np

N, C_IN, H, W = 32, 128, 56, 56
C_OUT, KH, KW = 256, 3, 3
N_CORES = 8
IMGS = N // N_CORES
HP, WP = H + 2, W + 2
RPT = 8
NT = H // RPT          # 7
TF = RPT * W           # 448
NH = C_OUT // 128      # 2

# x row-tiles: tag -> (row0, row1); tile t reads tag via T2TAG
XTILES = {"xa0": (0, 10), "xa1": (8, 18), "xaB": (16, 26), "xb": (24, 58)}
T2TAG = {0: "xa0", 1: "xa1", 2: "xaB", 3: "xb", 4: "xb", 5: "xb", 6: "xb"}
N_WARMUP_MM = 26

_CACHE = {}


def _build_program():
    import concourse.mybir as mybir
    import concourse.tile as tile
    from concourse import bacc

    F32 = mybir.dt.float32
    BF16 = mybir.dt.bfloat16

    nc = bacc.Bacc("TRN2", target_bir_lowering=False, debug=False,
                   enable_asserts=False)

    xp = nc.dram_tensor("xp", [IMGS, C_IN, HP, WP], BF16,
                        kind="ExternalInput").ap()
    w = nc.dram_tensor("w", [C_IN, KH * KW, C_OUT], BF16,
                       kind="ExternalInput").ap()
    b = nc.dram_tensor("b", [128, NH], F32, kind="ExternalInput").ap()
    out = nc.dram_tensor("out", [IMGS, C_OUT, H, W], BF16,
                         kind="ExternalOutput").ap()
    out_v = out.rearrange("n c a b -> n c (a b)")

    def xtile(pool, img, tag):
        r0, r1 = XTILES[tag]
        t = pool.tile([C_IN, r1 - r0, WP], BF16, tag=tag, name=f"x{img}_{tag}")
        return t, (lambda eng: eng.dma_start(out=t[:], in_=xp[img, :, r0:r1]))

    with tile.TileContext(nc) as tc:
        with (
            tc.tile_pool(name="consts", bufs=1) as consts,
            tc.tile_pool(name="xin", bufs=1) as xin,
            tc.tile_pool(name="outp", bufs=2) as outp,
            tc.tile_pool(name="psum", bufs=7, space="PSUM") as psum,
        ):
            # tiny scratch: the memset gating the first warmup matmul is
            # ~170ns instead of ~450ns, so the p-state ramp starts earlier
            scratch = consts.tile([128, 128], BF16, tag="scratch")
            nc.vector.memset(scratch[:], 0.0)

            # sync queue: 3 w k-group slices, then (later) out tiles
            w_sb = []
            for j in range(3):
                wj = consts.tile([C_IN, 3, C_OUT], BF16, tag=f"w{j}",
                                 name=f"w{j}")
                nc.sync.dma_start(out=wj[:], in_=w[:, 3 * j:3 * j + 3])
                w_sb.append(wj)

            # scalar queue: x row-tiles in consumption order, b after img0
            xts = {}
            b_sb = consts.tile([128, NH], F32, tag="b")
            for img in range(IMGS):
                xts[img] = {}
                for tag in XTILES:
                    t_, d_ = xtile(xin, img, tag)
                    d_(nc.scalar)
                    xts[img][tag] = t_
                if img == 0:
                    nc.scalar.dma_start(out=b_sb[:], in_=b)

            warm_ps = psum.tile([128, 128], F32, tag="warm", bufs=1)
            for _ in range(N_WARMUP_MM):
                nc.tensor.matmul(warm_ps[:, :], lhsT=scratch[:, :],
                                 rhs=scratch[:, :], start=True, stop=True)

            def mm(pt, src, r0, h, k, rows):
                kh, kw = divmod(k, KW)
                nc.tensor.matmul(
                    pt[:, :rows * W],
                    lhsT=w_sb[k // 3][:, k % 3, h * 128:(h + 1) * 128],
                    rhs=src[:, r0 + kh:r0 + kh + rows, kw:kw + W],
                    start=(k == 0),
                    stop=(k == KH * KW - 1),
                )

            def drain(ots, img, h, c0, pt, cols, dma_c0=None, dma_cols=None,
                      eng=None):
                nc.vector.tensor_scalar_add(
                    out=ots[h][:, c0:c0 + cols], in0=pt[:, :cols],
                    scalar1=b_sb[:, h:h + 1])
                if dma_cols is None:
                    dma_c0, dma_cols = c0, cols
                if dma_cols:
                    (eng or nc.sync).dma_start(
                        out=out_v[img, h * 128:(h + 1) * 128,
                                  dma_c0:dma_c0 + dma_cols],
                        in_=ots[h][:, dma_c0:dma_c0 + dma_cols])

            for img in range(IMGS):
                ots = [outp.tile([128, H * W], BF16, tag=f"ot{h}",
                                 name=f"ot{img}_{h}")
                       for h in range(NH)]
                for t in range(NT):
                    src = xts[img][T2TAG[t]]
                    r0 = RPT * t - XTILES[T2TAG[t]][0]
                    if t == 0:
                        # interleave h0/h1 k-groups: w1/w2 deadlines double
                        pts = [psum.tile([128, TF], F32, tag="pt",
                                          name=f"pt{img}_{t}_{h}")
                               for h in range(NH)]
                        for kc in range(3):
                            for h in range(NH):
                                for k in range(3 * kc, 3 * kc + 3):
                                    mm(pts[h], src, r0, h, k, RPT)
                        for h in range(NH):
                            drain(ots, img, h, t * TF, pts[h], TF)
                        continue
                    for h in range(NH):
                        last = (img == IMGS - 1 and t == NT - 1 and h == NH - 1)
                        # (row_offset, n_rows, taps); edge rows skip the taps
                        # that read the zero pad row
                        if t == NT - 1:
                            pieces = [(0, 7, range(9)), (7, 1, range(6))]
                        else:
                            pieces = [(0, RPT, range(9))]
                        for pi, (rr, rows, ks) in enumerate(pieces):
                            pt = psum.tile([128, rows * W], F32, tag="pt",
                                           name=f"pt{img}_{t}_{h}_{rr}")
                            for k in ks:
                                kh, kw = divmod(k, KW)
                                nc.tensor.matmul(
                                    pt[:, :rows * W],
                                    lhsT=w_sb[k // 3][:, k % 3,
                                              h * 128:(h + 1) * 128],
                                    rhs=src[:, r0 + rr + kh:
                                            r0 + rr + kh + rows, kw:kw + W],
                                    start=(k == ks[0]),
                                    stop=(k == ks[-1]),
                                )
                            if t == NT - 1:
                                # piece0: bias only; piece1: bias + one DMA
                                # covering the whole 448-col tile
                                dc0, dcols = (0, 0) if pi == 0 else (t * TF, TF)
                                drain(ots, img, h, t * TF + rr * W, pt,
                                      rows * W, dc0, dcols,
                                      nc.scalar if last else None)
                            else:
                                drain(ots, img, h, t * TF + rr * W, pt,
                                      rows * W)
    nc.compile()
    return nc


def get_program():
    if "nc" not in _CACHE:
        _CACHE["nc"] = _build_program()
    return _CACHE["nc"]


def make_in_maps(x, weight, bias):
    x = np.asarray(x, dtype=np.float32)
    weight = np.asarray(weight, dtype=np.float32)
    bias = np.asarray(bias, dtype=np.float32)

    xpad = np.zeros((N, C_IN, HP, WP), dtype=ml_dtypes.bfloat16)
    xpad[:, :, 1:1 + H, 1:1 + W] = x.astype(ml_dtypes.bfloat16)
    w_t = np.ascontiguousarray(
        weight.transpose(1, 2, 3, 0).reshape(C_IN, KH * KW, C_OUT)
    ).astype(ml_dtypes.bfloat16)
    b2 = np.ascontiguousarray(bias.reshape(NH, 128).T)

    return [
        {
            "xp": np.ascontiguousarray(xpad[i * IMGS:(i + 1) * IMGS]),
            "w": w_t,
            "b": b2,
        }
        for i in range(N_CORES)
    ]


def kernel(x, weight, bias):
    from concourse.bass_utils import run_bass_kernel_spmd

    nc = get_program()
    in_maps = make_in_maps(x, weight, bias)
    res = run_bass_kernel_spmd(nc, in_maps, core_ids=list(range(N_CORES)))
    return np.concatenate([res.results[i]["out"] for i in range(N_CORES)],
                          axis=0).astype(np.float32)


# revision 16
# speedup vs baseline: 1.0018x; 1.0018x over previous
"""v10: bf16 conv; jitter-robust head schedule.

Queue start order flips run to run (+-1us), so no tight cross-queue
deadlines: w loads as 3 k-group slices leading the sync queue (9 per-k
slices lost 5.6us to serialized ~620ns DMA triggers on the sync
engine); t=0 interleaves h0/h1 k-groups so w[3:6]/w[6:9] aren't needed
until ~1.1/2.2us after mm0. All x row-tiles + b lead the scalar queue
(xa00 first; only it gates mm0). Warmup 11 matmuls ~= the 3us p-state
ramp the PE needs after body entry anyway; any PE idle gap costs ~2x
(re-ramp), so warmup errs long.

v12: H-edge tap elision. Out row 55 only receives kh=0,1 (kh=2 reads
the zero pad row): t6 computes as a 7-row 9-tap piece + 1-row 6-tap
piece. v14: the two pieces drain as two bias-adds but ONE 448-col out
DMA (tiny trailing DMAs cost ~1.8us trigger-to-sem each); outputs are
bf16 (halved HBM write traffic/power), upcast to f32 on host.
"""

import sys

if "/opt/trn_rl_repo" not in sys.path:
    sys.path.insert(0, "/opt/trn_rl_repo")

import ml_dtypes
import numpy as np

N, C_IN, H, W = 32, 128, 56, 56
C_OUT, KH, KW = 256, 3, 3
N_CORES = 8
IMGS = N // N_CORES
HP, WP = H + 2, W + 2
RPT = 8
NT = H // RPT          # 7
TF = RPT * W           # 448
NH = C_OUT // 128      # 2

# x row-tiles: tag -> (row0, row1); tile t reads tag via T2TAG
XTILES = {"xa0": (0, 10), "xa1": (8, 18), "xaB": (16, 26), "xb": (24, 58)}
T2TAG = {0: "xa0", 1: "xa1", 2: "xaB", 3: "xb", 4: "xb", 5: "xb", 6: "xb"}
N_WARMUP_MM = 32

_CACHE = {}


def _build_program():
    import concourse.mybir as mybir
    import concourse.tile as tile
    from concourse import bacc

    F32 = mybir.dt.float32
    BF16 = mybir.dt.bfloat16

    nc = bacc.Bacc("TRN2", target_bir_lowering=False, debug=False,
                   enable_asserts=False)

    xp = nc.dram_tensor("xp", [IMGS, C_IN, HP, WP], BF16,
                        kind="ExternalInput").ap()
    w = nc.dram_tensor("w", [C_IN, KH * KW, C_OUT], BF16,
                       kind="ExternalInput").ap()
    b = nc.dram_tensor("b", [128, NH], F32, kind="ExternalInput").ap()
    out = nc.dram_tensor("out", [IMGS, C_OUT, H, W], BF16,
                         kind="ExternalOutput").ap()
    out_v = out.rearrange("n c a b -> n c (a b)")

    def xtile(pool, img, tag):
        r0, r1 = XTILES[tag]
        t = pool.tile([C_IN, r1 - r0, WP], BF16, tag=tag, name=f"x{img}_{tag}")
        return t, (lambda eng: eng.dma_start(out=t[:], in_=xp[img, :, r0:r1]))

    with tile.TileContext(nc) as tc:
        with (
            tc.tile_pool(name="consts", bufs=1) as consts,
            tc.tile_pool(name="xin", bufs=1) as xin,
            tc.tile_pool(name="outp", bufs=2) as outp,
            tc.tile_pool(name="psum", bufs=7, space="PSUM") as psum,
        ):
            # tiny scratch: the memset gating the first warmup matmul is
            # ~170ns instead of ~450ns, so the p-state ramp starts earlier
            scratch = consts.tile([128, 128], BF16, tag="scratch")
            nc.vector.memset(scratch[:], 0.0)

            # sync queue: 3 w k-group slices, then (later) out tiles
            w_sb = []
            for j in range(3):
                wj = consts.tile([C_IN, 3, C_OUT], BF16, tag=f"w{j}",
                                 name=f"w{j}")
                nc.sync.dma_start(out=wj[:], in_=w[:, 3 * j:3 * j + 3])
                w_sb.append(wj)

            # scalar queue: x row-tiles in consumption order, b after img0
            xts = {}
            b_sb = consts.tile([128, NH], F32, tag="b")
            for img in range(IMGS):
                xts[img] = {}
                for tag in XTILES:
                    t_, d_ = xtile(xin, img, tag)
                    d_(nc.scalar)
                    xts[img][tag] = t_
                if img == 0:
                    nc.scalar.dma_start(out=b_sb[:], in_=b)

            warm_ps = psum.tile([128, 128], F32, tag="warm", bufs=1)
            for _ in range(N_WARMUP_MM):
                nc.tensor.matmul(warm_ps[:, :], lhsT=scratch[:, :],
                                 rhs=scratch[:, :], start=True, stop=True)

            def mm(pt, src, r0, h, k, rows):
                kh, kw = divmod(k, KW)
                nc.tensor.matmul(
                    pt[:, :rows * W],
                    lhsT=w_sb[k // 3][:, k % 3, h * 128:(h + 1) * 128],
                    rhs=src[:, r0 + kh:r0 + kh + rows, kw:kw + W],
                    start=(k == 0),
                    stop=(k == KH * KW - 1),
                )

            def drain(ots, img, h, c0, pt, cols, dma_c0=None, dma_cols=None,
                      eng=None):
                nc.vector.tensor_scalar_add(
                    out=ots[h][:, c0:c0 + cols], in0=pt[:, :cols],
                    scalar1=b_sb[:, h:h + 1])
                if dma_cols is None:
                    dma_c0, dma_cols = c0, cols
                if dma_cols:
                    (eng or nc.sync).dma_start(
                        out=out_v[img, h * 128:(h + 1) * 128,
                                  dma_c0:dma_c0 + dma_cols],
                        in_=ots[h][:, dma_c0:dma_c0 + dma_cols])

            for img in range(IMGS):
                ots = [outp.tile([128, H * W], BF16, tag=f"ot{h}",
                                 name=f"ot{img}_{h}")
                       for h in range(NH)]
                for t in range(NT):
                    src = xts[img][T2TAG[t]]
                    r0 = RPT * t - XTILES[T2TAG[t]][0]
                    if t == 0:
                        # interleave h0/h1 k-groups: w1/w2 deadlines double
                        pts = [psum.tile([128, TF], F32, tag="pt",
                                          name=f"pt{img}_{t}_{h}")
                               for h in range(NH)]
                        for kc in range(3):
                            for h in range(NH):
                                for k in range(3 * kc, 3 * kc + 3):
                                    mm(pts[h], src, r0, h, k, RPT)
                        for h in range(NH):
                            drain(ots, img, h, t * TF, pts[h], TF)
                        continue
                    for h in range(NH):
                        last = (img == IMGS - 1 and t == NT - 1 and h == NH - 1)
                        # (row_offset, n_rows, taps); edge rows skip the taps
                        # that read the zero pad row
                        if t == NT - 1:
                            pieces = [(0, 7, range(9)), (7, 1, range(6))]
                        else:
                            pieces = [(0, RPT, range(9))]
                        for pi, (rr, rows, ks) in enumerate(pieces):
                            pt = psum.tile([128, rows * W], F32, tag="pt",
                                           name=f"pt{img}_{t}_{h}_{rr}")
                            for k in ks:
                                kh, kw = divmod(k, KW)
                                nc.tensor.matmul(
                                    pt[:, :rows * W],
                                    lhsT=w_sb[k // 3][:, k % 3,
                                              h * 128:(h + 1) * 128],
                                    rhs=src[:, r0 + rr + kh:
                                            r0 + rr + kh + rows, kw:kw + W],
                                    start=(k == ks[0]),
                                    stop=(k == ks[-1]),
                                )
                            if t == NT - 1:
                                # piece0: bias only; piece1: bias + one DMA
                                # covering the whole 448-col tile
                                dc0, dcols = (0, 0) if pi == 0 else (t * TF, TF)
                                drain(ots, img, h, t * TF + rr * W, pt,
                                      rows * W, dc0, dcols,
                                      nc.scalar if last else None)
                            else:
                                drain(ots, img, h, t * TF + rr * W, pt,
                                      rows * W)
    nc.compile()
    return nc


def get_program():
    if "nc" not in _CACHE:
        _CACHE["nc"] = _build_program()
    return _CACHE["nc"]


def make_in_maps(x, weight, bias):
    x = np.asarray(x, dtype=np.float32)
    weight = np.asarray(weight, dtype=np.float32)
    bias = np.asarray(bias, dtype=np.float32)

    xpad = np.zeros((N, C_IN, HP, WP), dtype=ml_dtypes.bfloat16)
    xpad[:, :, 1:1 + H, 1:1 + W] = x.astype(ml_dtypes.bfloat16)
    w_t = np.ascontiguousarray(
        weight.transpose(1, 2, 3, 0).reshape(C_IN, KH * KW, C_OUT)
    ).astype(ml_dtypes.bfloat16)
    b2 = np.ascontiguousarray(bias.reshape(NH, 128).T)

    return [
        {
            "xp": np.ascontiguousarray(xpad[i * IMGS:(i + 1) * IMGS]),
            "w": w_t,
            "b": b2,
        }
        for i in range(N_CORES)
    ]


def kernel(x, weight, bias):
    from concourse.bass_utils import run_bass_kernel_spmd

    nc = get_program()
    in_maps = make_in_maps(x, weight, bias)
    res = run_bass_kernel_spmd(nc, in_maps, core_ids=list(range(N_CORES)))
    return np.concatenate([res.results[i]["out"] for i in range(N_CORES)],
                          axis=0).astype(np.float32)


# revision 17
# speedup vs baseline: 1.0032x; 1.0014x over previous
"""v10: bf16 conv; jitter-robust head schedule.

Queue start order flips run to run (+-1us), so no tight cross-queue
deadlines: w loads as 3 k-group slices leading the sync queue (9 per-k
slices lost 5.6us to serialized ~620ns DMA triggers on the sync
engine); t=0 interleaves h0/h1 k-groups so w[3:6]/w[6:9] aren't needed
until ~1.1/2.2us after mm0. All x row-tiles + b lead the scalar queue
(xa00 first; only it gates mm0). Warmup 11 matmuls ~= the 3us p-state
ramp the PE needs after body entry anyway; any PE idle gap costs ~2x
(re-ramp), so warmup errs long.

v12: H-edge tap elision. Out row 55 only receives kh=0,1 (kh=2 reads
the zero pad row): t6 computes as a 7-row 9-tap piece + 1-row 6-tap
piece. v14: the two pieces drain as two bias-adds but ONE 448-col out
DMA (tiny trailing DMAs cost ~1.8us trigger-to-sem each); outputs are
bf16 (halved HBM write traffic/power), upcast to f32 on host.
"""

import sys

if "/opt/trn_rl_repo" not in sys.path:
    sys.path.insert(0, "/opt/trn_rl_repo")

import ml_dtypes
import numpy as np

N, C_IN, H, W = 32, 128, 56, 56
C_OUT, KH, KW = 256, 3, 3
N_CORES = 8
IMGS = N // N_CORES
HP, WP = H + 2, W + 2
RPT = 8
NT = H // RPT          # 7
TF = RPT * W           # 448
NH = C_OUT // 128      # 2

# x row-tiles: tag -> (row0, row1); tile t reads tag via T2TAG
XTILES = {"xa0": (0, 10), "xa1": (8, 18), "xaB": (16, 26), "xb": (24, 58)}
T2TAG = {0: "xa0", 1: "xa1", 2: "xaB", 3: "xb", 4: "xb", 5: "xb", 6: "xb"}
N_WARMUP_MM = 10

_CACHE = {}


def _build_program():
    import concourse.mybir as mybir
    import concourse.tile as tile
    from concourse import bacc

    F32 = mybir.dt.float32
    BF16 = mybir.dt.bfloat16

    nc = bacc.Bacc("TRN2", target_bir_lowering=False, debug=False,
                   enable_asserts=False)

    xp = nc.dram_tensor("xp", [IMGS, C_IN, HP, WP], BF16,
                        kind="ExternalInput").ap()
    w = nc.dram_tensor("w", [C_IN, KH * KW, C_OUT], BF16,
                       kind="ExternalInput").ap()
    b = nc.dram_tensor("b", [128, NH], F32, kind="ExternalInput").ap()
    out = nc.dram_tensor("out", [IMGS, C_OUT, H, W], BF16,
                         kind="ExternalOutput").ap()
    out_v = out.rearrange("n c a b -> n c (a b)")

    def xtile(pool, img, tag):
        r0, r1 = XTILES[tag]
        t = pool.tile([C_IN, r1 - r0, WP], BF16, tag=tag, name=f"x{img}_{tag}")
        return t, (lambda eng: eng.dma_start(out=t[:], in_=xp[img, :, r0:r1]))

    with tile.TileContext(nc) as tc:
        with (
            tc.tile_pool(name="consts", bufs=1) as consts,
            tc.tile_pool(name="xin", bufs=1) as xin,
            tc.tile_pool(name="outp", bufs=2) as outp,
            tc.tile_pool(name="psum", bufs=7, space="PSUM") as psum,
        ):
            scratch = consts.tile([128, TF], BF16, tag="scratch")
            nc.vector.memset(scratch[:], 0.0)

            # sync queue: 3 w k-group slices, then (later) out tiles
            w_sb = []
            for j in range(3):
                wj = consts.tile([C_IN, 3, C_OUT], BF16, tag=f"w{j}",
                                 name=f"w{j}")
                nc.sync.dma_start(out=wj[:], in_=w[:, 3 * j:3 * j + 3])
                w_sb.append(wj)

            # scalar queue: x row-tiles in consumption order, b after img0
            xts = {}
            b_sb = consts.tile([128, NH], F32, tag="b")
            for img in range(IMGS):
                xts[img] = {}
                for tag in XTILES:
                    t_, d_ = xtile(xin, img, tag)
                    d_(nc.scalar)
                    xts[img][tag] = t_
                if img == 0:
                    nc.scalar.dma_start(out=b_sb[:], in_=b)

            warm_ps = psum.tile([128, TF], F32, tag="warm", bufs=1)
            for _ in range(N_WARMUP_MM):
                nc.tensor.matmul(warm_ps[:, :], lhsT=scratch[:, :128],
                                 rhs=scratch[:, :], start=True, stop=True)

            def mm(pt, src, r0, h, k, rows):
                kh, kw = divmod(k, KW)
                nc.tensor.matmul(
                    pt[:, :rows * W],
                    lhsT=w_sb[k // 3][:, k % 3, h * 128:(h + 1) * 128],
                    rhs=src[:, r0 + kh:r0 + kh + rows, kw:kw + W],
                    start=(k == 0),
                    stop=(k == KH * KW - 1),
                )

            def drain(ots, img, h, c0, pt, cols, dma_c0=None, dma_cols=None,
                      eng=None):
                nc.vector.tensor_scalar_add(
                    out=ots[h][:, c0:c0 + cols], in0=pt[:, :cols],
                    scalar1=b_sb[:, h:h + 1])
                if dma_cols is None:
                    dma_c0, dma_cols = c0, cols
                if dma_cols:
                    (eng or nc.sync).dma_start(
                        out=out_v[img, h * 128:(h + 1) * 128,
                                  dma_c0:dma_c0 + dma_cols],
                        in_=ots[h][:, dma_c0:dma_c0 + dma_cols])

            for img in range(IMGS):
                ots = [outp.tile([128, H * W], BF16, tag=f"ot{h}",
                                 name=f"ot{img}_{h}")
                       for h in range(NH)]
                for t in range(NT):
                    src = xts[img][T2TAG[t]]
                    r0 = RPT * t - XTILES[T2TAG[t]][0]
                    if t == 0:
                        # interleave h0/h1 k-groups: w1/w2 deadlines double
                        pts = [psum.tile([128, TF], F32, tag="pt",
                                          name=f"pt{img}_{t}_{h}")
                               for h in range(NH)]
                        for kc in range(3):
                            for h in range(NH):
                                for k in range(3 * kc, 3 * kc + 3):
                                    mm(pts[h], src, r0, h, k, RPT)
                        for h in range(NH):
                            drain(ots, img, h, t * TF, pts[h], TF)
                        continue
                    for h in range(NH):
                        last = (img == IMGS - 1 and t == NT - 1 and h == NH - 1)
                        # (row_offset, n_rows, taps); edge rows skip the taps
                        # that read the zero pad row
                        if t == NT - 1:
                            pieces = [(0, 7, range(9)), (7, 1, range(6))]
                        else:
                            pieces = [(0, RPT, range(9))]
                        for pi, (rr, rows, ks) in enumerate(pieces):
                            pt = psum.tile([128, rows * W], F32, tag="pt",
                                           name=f"pt{img}_{t}_{h}_{rr}")
                            for k in ks:
                                kh, kw = divmod(k, KW)
                                nc.tensor.matmul(
                                    pt[:, :rows * W],
                                    lhsT=w_sb[k // 3][:, k % 3,
                                              h * 128:(h + 1) * 128],
                                    rhs=src[:, r0 + rr + kh:
                                            r0 + rr + kh + rows, kw:kw + W],
                                    start=(k == ks[0]),
                                    stop=(k == ks[-1]),
                                )
                            if t == NT - 1 and not last:
                                # piece0: bias only; piece1: bias + one DMA
                                # covering the whole 448-col tile
                                dc0, dcols = (0, 0) if pi == 0 else (t * TF, TF)
                                drain(ots, img, h, t * TF + rr * W, pt,
                                      rows * W, dc0, dcols)
                            elif last:
                                # final tile: bias per piece, then the 448
                                # cols leave as two 224-col DMAs on separate
                                # trigger engines (parallel trigger+data+sem)
                                drain(ots, img, h, t * TF + rr * W, pt,
                                      rows * W, 0, 0)
                                if pi == 1:
                                    HTF = TF // 2
                                    nc.scalar.dma_start(
                                        out=out_v[img, h * 128:(h + 1) * 128,
                                                  t * TF:t * TF + HTF],
                                        in_=ots[h][:, t * TF:t * TF + HTF])
                                    nc.sync.dma_start(
                                        out=out_v[img, h * 128:(h + 1) * 128,
                                                  t * TF + HTF:(t + 1) * TF],
                                        in_=ots[h][:, t * TF + HTF:(t + 1) * TF])
                            else:
                                drain(ots, img, h, t * TF + rr * W, pt,
                                      rows * W)
    nc.compile()
    return nc


def get_program():
    if "nc" not in _CACHE:
        _CACHE["nc"] = _build_program()
    return _CACHE["nc"]


def make_in_maps(x, weight, bias):
    x = np.asarray(x, dtype=np.float32)
    weight = np.asarray(weight, dtype=np.float32)
    bias = np.asarray(bias, dtype=np.float32)

    xpad = np.zeros((N, C_IN, HP, WP), dtype=ml_dtypes.bfloat16)
    xpad[:, :, 1:1 + H, 1:1 + W] = x.astype(ml_dtypes.bfloat16)
    w_t = np.ascontiguousarray(
        weight.transpose(1, 2, 3, 0).reshape(C_IN, KH * KW, C_OUT)
    ).astype(ml_dtypes.bfloat16)
    b2 = np.ascontiguousarray(bias.reshape(NH, 128).T)

    return [
        {
            "xp": np.ascontiguousarray(xpad[i * IMGS:(i + 1) * IMGS]),
            "w": w_t,
            "b": b2,
        }
        for i in range(N_CORES)
    ]


def kernel(x, weight, bias):
    from concourse.bass_utils import run_bass_kernel_spmd

    nc = get_program()
    in_maps = make_in_maps(x, weight, bias)
    res = run_bass_kernel_spmd(nc, in_maps, core_ids=list(range(N_CORES)))
    return np.concatenate([res.results[i]["out"] for i in range(N_CORES)],
                          axis=0).astype(np.float32)
